# revision 76
# baseline (speedup 1.0000x reference)
"""Trainium2 Bass kernel for nn_LogLinearAttention (B=2,T=1024,Dm=1024,H=16,D=64,L=12).

Math (validated numerically against the jax reference):
  out = ((S*Mw)@V / rowsum(S*Mw)) @ ow + ob   with S = phi(xQ) phi(xK)^T,
  Mw[i,j] = w[i, lev(i,j)],  lev(i,j) = msb((i+1) XOR j)  (0-based, j<=i).
Softmax over levels cancels in num/den, so w~ = exp(logits) is used raw.
phi(a) = elu(a)+1 = max(a + 1, min(exp(a), 1)).

v2: all matmul operands bf16 (tolerance 2e-2 >> bf16 noise), K projected once
(token orientation) and transposed on-chip, batched per-(b,h) wide mask/score
ops, elementwise spread over DVE/Act/Pool, output DMAd fp32 straight from
PSUM, projections chunk-pipelined against the input DMA, and batch-0
attention emitted interleaved with batch-1 projections.

Sharding: 8 cores, core c owns heads {2c, 2c+1} for both batches
(tensor-parallel projections, head-parallel attention, partial output
projections summed on host).
"""

from contextlib import ExitStack

import numpy as np
import ml_dtypes

import concourse.bass as bass
import concourse.tile as tile
import concourse.mybir as mybir
from concourse import bacc
from concourse.bass_utils import run_bass_kernel_spmd
from concourse.masks import make_identity

F32 = mybir.dt.float32
BF16 = mybir.dt.bfloat16
U8 = mybir.dt.uint8

B, T, DM, H, D, L = 2, 1024, 1024, 16, 64, 12
C = 128            # token block
NB = T // C        # 8
NCORES = 8
NTB = B * T // C   # 16 token blocks over (b, t)
KC = DM // 128     # 8 contraction chunks
# wall layout: qw 128 | V-pack 132 | kw 128 | lw-pack 24
WALLN = 412
PKW = 160          # pk per-tb width: ek 128 | wt 24 | pad 8

AF = mybir.ActivationFunctionType
ALU = mybir.AluOpType


def _msb(v):
    return v.bit_length() - 1


def _decomp(bi):
    """Fenwick decomposition of block-prefix [0, bi): [(beta, size, g), ...]."""
    segs, start = [], 0
    for g in range(7, -1, -1):
        if (bi >> g) & 1:
            segs.append((start, 1 << g, g))
            start += 1 << g
    return segs


# state-tile layout: leaves P0..P6 at slots 0..6; combined segments:
_COMB = {(0, 2): 7, (0, 4): 8, (4, 2): 9}


def _l127(bi):
    return 7 + _msb((bi + 1) ^ bi)


def _build_slot_consts():
    """Shared COLIND/ROWIND [128,128] and REPLT [12,128] (slot-127 zeroed)."""
    colind = np.zeros((128, C), np.float32)
    rowind = np.zeros((128, C), np.float32)
    replt = np.zeros((L, 128), np.float32)
    i1 = np.arange(1, C + 1)
    slot = 0
    for c in range(7):
        for m in range(1 << (6 - c)):
            rows = (((i1 >> (c + 1)) == m) & (((i1 >> c) & 1) == 1) & (i1 < C))
            rowind[slot, :] = rows.astype(np.float32)
            colind[slot, m * (1 << (c + 1)): m * (1 << (c + 1)) + (1 << c)] = 1.0
            replt[c, slot] = 1.0
            slot += 1
    assert slot == 127
    rowind[127, 127] = 1.0
    colind[127, :] = 1.0
    # replt slot-127 column stays zero; the row-127 value is patched into
    # wrow[127, bi, 127] at runtime.
    return colind, rowind, replt


def _w_fixups():
    """Row-127 level remaps on w~ for inter scale columns: [(bi, tgt, src)]."""
    fixes = []
    for bi in range(NB):
        for (beta, size, g) in _decomp(bi):
            tgt, src = 7 + g, 7 + _msb((bi + 1) ^ beta)
            if src != tgt:
                fixes.append((bi, tgt, src))
    return fixes


_PROGRAM_CACHE = {}


def _build_program(with_o1_bias: bool):
    nc = bacc.Bacc(trn_type="TRN2", target_bir_lowering=False, debug=False,
                   num_devices=NCORES)

    xT = nc.dram_tensor("xT", [DM, B * T], BF16, kind="ExternalInput").ap()
    wall = nc.dram_tensor("wall", [DM, WALLN], BF16, kind="ExternalInput").ap()
    owd = nc.dram_tensor("owd", [128, 1024], BF16, kind="ExternalInput").ap()
    # colrow: colind 128 | rowind-tiled 1024
    colrow = nc.dram_tensor("colrow", [128, 1152], BF16, kind="ExternalInput").ap()
    replt_d = nc.dram_tensor("replt", [32, 2 * 128], BF16, kind="ExternalInput").ap()
    # biasf: qb | qb+1 | msk127
    biasf_d = nc.dram_tensor("biasf", [128, 3], F32, kind="ExternalInput").ap()
    m127 = nc.dram_tensor("m127", [128, 1], U8, kind="ExternalInput").ap()
    bias1 = nc.dram_tensor("bias1", [128, 284], F32, kind="ExternalInput").ap()
    out_d = nc.dram_tensor("out", [B * T, DM], BF16, kind="ExternalOutput").ap()

    fixes = _w_fixups()

    with tile.TileContext(nc) as tc, ExitStack() as ctx:
        const = ctx.enter_context(tc.tile_pool(name="const", bufs=1))
        big = ctx.enter_context(tc.tile_pool(name="big", bufs=1))
        sm = ctx.enter_context(tc.tile_pool(name="sm", bufs=3))
        smd = ctx.enter_context(tc.tile_pool(name="smd", bufs=3))

        # ---------- input DMAs: wall + x chunks interleaved ----------
        wall_sb = const.tile([128, KC, WALLN], BF16)
        xch = big.tile([128, KC, B * T], BF16)
        for k in range(KC):
            nc.sync.dma_start(out=wall_sb[:, k, :],
                              in_=wall[128 * k:128 * (k + 1), :])
            nc.sync.dma_start(out=xch[:, k, :],
                              in_=xT[128 * k:128 * (k + 1), :])
        colrow_sb = const.tile([128, 1152], BF16)
        replt_sb = const.tile([32, 2, 128], BF16)
        biasf_sb = const.tile([128, 3], F32)
        m127_sb = const.tile([128, 1], U8)
        ow_sb = const.tile([128, 1024], BF16)
        nc.sync.dma_start(out=colrow_sb, in_=colrow)
        nc.sync.dma_start(out=replt_sb, in_=replt_d)
        nc.sync.dma_start(out=biasf_sb, in_=biasf_d)
        nc.sync.dma_start(out=m127_sb, in_=m127)
        nc.sync.dma_start(out=ow_sb, in_=owd)
        if with_o1_bias:
            bias1_sb = const.tile([128, 284], F32)
            nc.sync.dma_start(out=bias1_sb, in_=bias1)
        ident = const.tile([128, 128], BF16)
        make_identity(nc, ident)
        colind_sb = colrow_sb[:, 0:128]
        rowind_sb = colrow_sb[:, 128:1152]

        QpT = big.tile([128, B * T], BF16)
        KpT = big.tile([128, B * T], BF16)
        Kp1 = big.tile([128, NTB, 128], BF16)
        Vp1 = big.tile([128, NTB, 132], BF16)
        pk = big.tile([128, NTB, PKW], BF16)   # ek 0:128 | wt 128:152 | pad
        wtT = big.tile([32, NTB * 128], BF16)
        attn_a = big.tile([128, NTB, 128], BF16)
        attnT = big.tile([128, T], BF16)

        # pad cols of pk must be defined before the wtT transposes read them
        nc.gpsimd.memset(pk[:, :, 152:160], 0.0)
        zero66 = const.tile([128, 66], BF16)
        nc.gpsimd.memset(zero66, 0.0)

        def o2_mm(pt, sl, k):
            nc.tensor.matmul(pt, wall_sb[:, k, 0:128],
                             xch[:, k, 512 * sl:512 * (sl + 1)],
                             start=(k == 0), stop=(k == KC - 1))

        def o1_mm(pt, tb, k):
            nc.tensor.matmul(pt, xch[:, k, 128 * tb:128 * (tb + 1)],
                             wall_sb[:, k, 128:412],
                             start=(k == 0), stop=(k == KC - 1))

        def o2_fin(pt, sl):
            # phi on Q -> QpT slice
            et = sm.tile([128, 512], BF16, tag="o2e", bufs=2)
            nc.scalar.activation(et, pt, AF.Exp, bias=biasf_sb[:, 0:1])
            ec = sm.tile([128, 512], BF16, tag="o2c", bufs=2)
            nc.vector.tensor_scalar_min(out=ec, in0=et, scalar1=1.0)
            nc.vector.scalar_tensor_tensor(
                out=QpT[:, 512 * sl:512 * (sl + 1)], in0=pt,
                scalar=biasf_sb[:, 1:2], in1=ec, op0=ALU.add, op1=ALU.max)

        def o1_fin(pt, tb):
            if with_o1_bias:
                nc.vector.tensor_add(pt, pt, bias1_sb)
            # exp over K|l cols in one op -> pk (K-exp 0:128, w~ 128:152)
            nc.scalar.activation(pk[:, tb, 0:152], pt[:, 132:284], AF.Exp)
            ec = sm.tile([128, 128], BF16, tag="o1c", bufs=4)
            nc.vector.tensor_scalar_min(out=ec, in0=pk[:, tb, 0:128],
                                        scalar1=1.0)
            nc.vector.scalar_tensor_tensor(
                out=Kp1[:, tb, :], in0=pt[:, 132:260], scalar=1.0,
                in1=ec, op0=ALU.add, op1=ALU.max)
            nc.scalar.copy(Vp1[:, tb, :], pt[:, 0:132])

        fix_by_bi = {}
        for (bi, tgt, srcl) in fixes:
            fix_by_bi.setdefault(bi, []).append((tgt, srcl))

        def tb_fin(tb, trpool):
            """Per-tb epilogue: row-127 fixes, l127 column, wtT + KpT."""
            for (tgt, srcl) in fix_by_bi.get(tb % NB, []):
                def _wcols(col):
                    return bass.AP(
                        tensor=pk.tensor,
                        offset=pk.offset + tb * PKW + 128 + col,
                        ap=[[NTB * PKW, 128], [12, 2]])
                mk = bass.AP(tensor=m127_sb.tensor, offset=m127_sb.offset,
                             ap=[[1, 128], [0, 2]])
                nc.vector.copy_predicated(out=_wcols(tgt), mask=mk,
                                          data=_wcols(srcl))
            # l127-selected w~ column per head -> pk cols 152/153 (wtT rows
            # 24/25, routed to slot 127 by the extended replt constant)
            lc = 128 + _l127(tb % NB)
            src = bass.AP(tensor=pk.tensor, offset=pk.offset + tb * PKW + lc,
                          ap=[[NTB * PKW, 128], [12, 2]])
            dst = bass.AP(tensor=pk.tensor, offset=pk.offset + tb * PKW + 152,
                          ap=[[NTB * PKW, 128], [1, 2]])
            nc.vector.tensor_copy(dst, src)
            # wtT transpose (dedicated exact-shape psum tiles: a transpose
            # writing a partition-subview of a larger tile wedges the core)
            pt = trpool.tile([32, 128], BF16, tag="wtr", bufs=1,
                             name=f"wtt{tb}")
            nc.tensor.transpose(pt, pk[:, tb, 128:160], ident)
            nc.scalar.copy(wtT[:, 128 * tb:128 * (tb + 1)], pt)


        def v_ones(b):
            v4 = Vp1.rearrange("p b (two ss) -> p b two ss", two=2, ss=66)
            nc.gpsimd.memset(v4[:, NB * b:NB * (b + 1), :, 64:65], 1.0)

        def states(b, pspool, wbufs=2):
            ST = smd.tile([128, 10, 132], BF16, tag="ST", bufs=2)
            for beta in range(7):
                blk = b * NB + beta
                pp = pspool.tile([128, NB * C], F32, tag="wide", bufs=wbufs,
                                 name=f"st{beta}")
                nc.tensor.matmul(pp[:, 0:132], Kp1[:, blk, :], Vp1[:, blk, :],
                                 start=True, stop=True)
                nc.scalar.copy(ST[:, beta, :], pp[:, 0:132])
            nc.vector.tensor_add(ST[:, 7, :], ST[:, 0, :], ST[:, 1, :])
            nc.vector.tensor_add(ST[:, 8, :], ST[:, 7, :], ST[:, 2, :])
            nc.vector.tensor_add(ST[:, 8, :], ST[:, 8, :], ST[:, 3, :])
            nc.vector.tensor_add(ST[:, 9, :], ST[:, 4, :], ST[:, 5, :])
            return ST

        def kp_dmat(lo, n):
            in_ = Kp1[:, lo:lo + n, :]
            out = bass.AP(tensor=KpT.tensor, offset=KpT.offset + 128 * lo,
                          ap=[[2048, 128], [128, n], [1, 128]])
            nc.sync.dma_start_transpose(out, in_)

        def at_dmat(b, lo, n):
            in_ = attn_a[:, NB * b + lo:NB * b + lo + n, :]
            out = bass.AP(tensor=attnT.tensor, offset=attnT.offset + 128 * lo,
                          ap=[[1024, 128], [128, n], [1, 128]])
            nc.sync.dma_start_transpose(out, in_)

        def out_blk(b, bi, pspool, ktrbufs=2, wbufs=2, act_only=False):
            blk = b * NB + bi
            pt = pspool.tile([128, 128], BF16, tag="ktr", bufs=ktrbufs)
            nc.tensor.transpose(pt, attn_a[:, blk, :], ident)
            if act_only:
                nc.scalar.copy(attnT[:, 128 * bi:128 * (bi + 1)], pt)
            else:
                nc.vector.tensor_copy(attnT[:, 128 * bi:128 * (bi + 1)], pt)
            po = pspool.tile([128, 1024], F32, tag="wide", bufs=wbufs)
            ot = sm.tile([128, 1024], BF16, tag="ot", bufs=4)
            for half in range(2):
                nc.tensor.matmul(
                    po[:, 512 * half:512 * (half + 1)],
                    attnT[:, 128 * bi:128 * (bi + 1)],
                    ow_sb[:, 512 * half:512 * (half + 1)],
                    start=True, stop=True, skip_group_check=True)
            if act_only or bi % 2 == 0:
                nc.scalar.copy(ot, po)
                nc.sync.dma_start(out=out_d[C * blk:C * (blk + 1), :], in_=ot)
            else:
                nc.scalar.copy(ot[:, 0:512], po[:, 0:512])
                nc.vector.tensor_copy(ot[:, 512:1024], po[:, 512:1024])
                for hf in range(2):
                    nc.sync.dma_start(
                        out=out_d[C * blk:C * (blk + 1),
                                  512 * hf:512 * (hf + 1)],
                        in_=ot[:, 512 * hf:512 * (hf + 1)])

        wbh = [2]   # wide-ring bufs holder (set per attention scope)

        def stage_A(b, h, ST, pspool):
            """Mask + scores for (b,h): returns smdt (SBUF bf16)."""
            hp = slice(64 * h, 64 * (h + 1))
            wr = pspool.tile([128, NB * C], F32, tag="wide", bufs=wbh[0],
                             name=f"wr{b}{h}")
            for q in range(2):
                nc.tensor.matmul(
                    wr[:, 512 * q:512 * (q + 1)], replt_sb[:, h, :],
                    wtT[:, 1024 * b + 512 * q:1024 * b + 512 * (q + 1)],
                    start=True, stop=True, skip_group_check=True)
            # scores immediately after wr on PE; wrow (DVE) and the sdt->SBUF
            # drain (Act) overlap them, so the serial chain is just
            # wr -> wrow -> mw -> smdt.
            sdt = pspool.tile([128, NB * C], F32, tag="wide", bufs=wbh[0],
                              name=f"sdt{b}{h}")
            for bi in range(NB):
                tok = slice(C * (b * NB + bi), C * (b * NB + bi + 1))
                nc.tensor.matmul(sdt[:, 128 * bi:128 * (bi + 1)],
                                 KpT[hp, tok], QpT[hp, tok],
                                 start=True, stop=True, skip_group_check=True)
            # halves pipeline: wrow/sdsb/mw/smdt split per 512-col half so
            # stage_B's first half starts ~2us earlier.
            wrow = smd.tile([128, NB * C], BF16, tag="wrow", bufs=4,
                            name=f"wrow{b}{h}")
            sdsb = smd.tile([128, NB * C], BF16, tag="mwsb", bufs=4,
                            name=f"sdsb{b}{h}")
            mw = pspool.tile([128, NB * C], F32, tag="wide", bufs=wbh[0],
                             name=f"mw{b}{h}")
            smdt = smd.tile([128, NB * C], BF16, tag="smdt", bufs=4,
                            name=f"smdt{b}{h}")
            for q in range(2):
                sl = slice(512 * q, 512 * (q + 1))
                nc.vector.tensor_mul(wrow[:, sl], wr[:, sl], rowind_sb[:, sl])
                nc.scalar.copy(sdsb[:, sl], sdt[:, sl])
                nc.tensor.matmul(mw[:, sl], colind_sb, wrow[:, sl],
                                 start=True, stop=True, skip_group_check=True)
                nc.vector.tensor_mul(smdt[:, sl], mw[:, sl], sdsb[:, sl])
            return smdt

        def stage_B(b, h, half, ST, smdt, num, pspool):
            """Numerators for 4 blocks of (b,h): batched matmuls then dense
            DVE combine chains."""
            hp = slice(64 * h, 64 * (h + 1))
            vc = slice(66 * h, 66 * (h + 1))
            bis = range(4 * half, 4 * half + 4)
            W = pspool.tile([128, NB * C], F32, tag="wide", bufs=wbh[0],
                            name=f"W{b}{h}{half}")
            slot_n = 0

            def wslot():
                nonlocal slot_n
                cc = (slot_n // 7) * 512 + (slot_n % 7) * 66
                slot_n += 1
                return cc
            col = {}
            for bi in bis:
                blk = b * NB + bi
                col[bi] = wslot()
                nc.tensor.matmul(
                    W[:, col[bi]:col[bi] + 66],
                    smdt[:, 128 * bi:128 * (bi + 1)],
                    Vp1[:, blk, vc], start=True, stop=True,
                    skip_group_check=True)
            scol = {}
            for bi in bis:
                blk = b * NB + bi
                tok = slice(C * blk, C * (blk + 1))
                for si, (beta, sz, g) in enumerate(_decomp(bi)):
                    slot = beta if sz == 1 else _COMB[(beta, sz)]
                    cc = wslot()
                    scol[(bi, si)] = (cc, g)
                    nc.tensor.matmul(
                        W[:, cc:cc + 66], QpT[hp, tok],
                        ST[hp, slot, vc], start=True, stop=True,
                        skip_group_check=True)
            for bi in bis:
                if not _decomp(bi):
                    nc.scalar.copy(num[:, bi, :], W[:, col[bi]:col[bi] + 66])
            for bi in bis:
                blk = b * NB + bi
                segs = _decomp(bi)
                for si in range(len(segs)):
                    cc, g = scol[(bi, si)]
                    sc = pk[:, blk,
                            128 + 12 * h + 7 + g:128 + 12 * h + 8 + g]
                    if si == 0:
                        # seed the chain on DVE directly (no Act handoff)
                        nc.vector.scalar_tensor_tensor(
                            out=num[:, bi, :], in0=W[:, cc:cc + 66],
                            scalar=sc, in1=zero66,
                            op0=ALU.mult, op1=ALU.add)
                    else:
                        nc.vector.scalar_tensor_tensor(
                            out=num[:, bi, :], in0=W[:, cc:cc + 66],
                            scalar=sc, in1=num[:, bi, :],
                            op0=ALU.mult, op1=ALU.add)
                if segs:
                    nc.vector.tensor_add(num[:, bi, :],
                                         W[:, col[bi]:col[bi] + 66],
                                         num[:, bi, :])

        def stage_div(b, h, num, lo=0, n=NB, eng=None):
            rec = smd.tile([128, NB], F32, tag="rec", bufs=4)
            den = bass.AP(tensor=num.tensor, offset=num.offset + 66 * lo + 64,
                          ap=[[NB * 66, 128], [66, n]])
            nc.vector.reciprocal(rec[:, 0:n], den)
            dst = bass.AP(tensor=attn_a.tensor,
                          offset=attn_a.offset + (b * NB + lo) * 128 + 64 * h,
                          ap=[[NTB * 128, 128], [128, n], [1, 64]])
            n0 = bass.AP(tensor=num.tensor, offset=num.offset + 66 * lo,
                         ap=[[NB * 66, 128], [66, n], [1, 64]])
            rc = bass.AP(tensor=rec.tensor, offset=rec.offset,
                         ap=[[NB, 128], [1, n], [0, 64]])
            (eng or nc.vector).tensor_tensor(out=dst, in0=n0, in1=rc,
                                             op=ALU.mult)

        # ================= scope 1: batch-0 projections + o2 =================
        with tc.tile_pool(name="ps1", bufs=1, space="PSUM") as ps1:
            waves = [
                [("o2", 0), ("o2", 1), ("o2", 2), ("o1", 0), ("o1", 1),
                 ("o1", 2), ("o1", 3)],
                [("o2", 3), ("o1", 4), ("o1", 5), ("o1", 6), ("o1", 7)],
            ]
            for wi, wave in enumerate(waves):
                tiles = {}
                for kind, idx in wave:
                    if kind == "o2":
                        tiles[(kind, idx)] = ps1.tile(
                            [128, 512], F32, tag="o2", bufs=3,
                            name=f"o2_{idx}")
                    else:
                        tiles[(kind, idx)] = ps1.tile(
                            [128, 284], F32, tag="o1", bufs=4,
                            name=f"o1_{idx}")
                for k in range(KC):
                    for kind, idx in wave:
                        if kind == "o2":
                            o2_mm(tiles[(kind, idx)], idx, k)
                        else:
                            o1_mm(tiles[(kind, idx)], idx, k)
                for kind, idx in wave:
                    if kind == "o2":
                        o2_fin(tiles[(kind, idx)], idx)
                    else:
                        o1_fin(tiles[(kind, idx)], idx)
                        tb_fin(idx, ps1)
                o1s = [i for k, i in wave if k == "o1"]
                if o1s:
                    kp_dmat(min(o1s), len(o1s))
            v_ones(0)

        # == scope 2: b0 attention zipped with b1 projections (PE filler) ==
        with tc.tile_pool(name="ps2", bufs=1, space="PSUM") as ps2:
            def b1_wave(tbs):
                tiles = {}
                for tb in tbs:
                    tiles[tb] = ps2.tile([128, 284], F32, tag="o1b", bufs=2,
                                         name=f"o1b{tb}")
                for k in range(KC):
                    for tb in tbs:
                        o1_mm(tiles[tb], tb, k)
                for tb in tbs:
                    o1_fin(tiles[tb], tb)
                    tb_fin(tb, ps2)
                kp_dmat(min(tbs), len(tbs))

            ST0 = states(0, ps2, wbufs=2)
            num00 = smd.tile([128, NB, 66], BF16, tag="num", bufs=6,
                             name="num00")
            num01 = smd.tile([128, NB, 66], BF16, tag="num", bufs=6,
                             name="num01")
            wbh[0] = 2
            sm00 = stage_A(0, 0, ST0, ps2)
            b1_wave((8, 9))
            stage_B(0, 0, 0, ST0, sm00, num00, ps2)
            stage_B(0, 0, 1, ST0, sm00, num00, ps2)
            stage_div(0, 0, num00)
            sm01 = stage_A(0, 1, ST0, ps2)
            b1_wave((10, 11))
            stage_B(0, 1, 0, ST0, sm01, num01, ps2)
            stage_B(0, 1, 1, ST0, sm01, num01, ps2)
            stage_div(0, 1, num01)
            b1_wave((12, 13))
            b1_wave((14, 15))
            v_ones(1)

        # ====== scope 3: b1 attention + b1 outputs ========================
        with tc.tile_pool(name="ps3", bufs=1, space="PSUM") as ps3:
            wbh[0] = 3
            ST1 = states(1, ps3, wbufs=3)
            num10 = smd.tile([128, NB, 66], BF16, tag="num", bufs=6,
                             name="num10")
            num11 = smd.tile([128, NB, 66], BF16, tag="num", bufs=6,
                             name="num11")
            sm10 = stage_A(1, 0, ST1, ps3)
            sm11 = stage_A(1, 1, ST1, ps3)
            stage_B(1, 0, 0, ST1, sm10, num10, ps3)
            for bi in range(0, 4):
                out_blk(0, bi, ps3, wbufs=3, act_only=True)
            stage_B(1, 0, 1, ST1, sm10, num10, ps3)
            stage_div(1, 0, num10, eng=nc.gpsimd)
            for bi in range(4, 8):
                out_blk(0, bi, ps3, wbufs=3, act_only=True)
            stage_B(1, 1, 0, ST1, sm11, num11, ps3)
            stage_div(1, 1, num11, lo=0, n=4)
            for bi in range(0, 4):
                out_blk(1, bi, ps3, wbufs=3)
            stage_B(1, 1, 1, ST1, sm11, num11, ps3)
            stage_div(1, 1, num11, lo=4, n=2)
            for bi in range(4, 6):
                out_blk(1, bi, ps3, wbufs=3)
            stage_div(1, 1, num11, lo=6, n=1)
            out_blk(1, 6, ps3, wbufs=3)
            stage_div(1, 1, num11, lo=7, n=1)
            out_blk(1, 7, ps3, wbufs=3)

    nc.compile()
    return nc


def _host_prep(inputs):
    bf = ml_dtypes.bfloat16
    x = np.ascontiguousarray(
        np.asarray(inputs["x"], np.float32).reshape(B * T, DM))
    xT = np.ascontiguousarray(x.T.astype(bf))
    qw = np.asarray(inputs["qw"], np.float32)
    kw = np.asarray(inputs["kw"], np.float32)
    vw = np.asarray(inputs["vw"], np.float32)
    lw = np.asarray(inputs["lw"], np.float32)
    ow = np.asarray(inputs["ow"], np.float32)
    qb = np.asarray(inputs["qb"], np.float32)
    kb = np.asarray(inputs["kb"], np.float32)
    vb = np.asarray(inputs["vb"], np.float32)
    lb = np.asarray(inputs["lb"], np.float32)

    colind, rowind, replt = _build_slot_consts()
    replt2 = np.zeros((32, 2 * 128), np.float32)
    replt2[0:12, 0:128] = replt          # h0 variant: levels at rows 0:12
    replt2[12:24, 128:256] = replt       # h1 variant: levels at rows 12:24
    # wtT rows 24/25 carry the l127-remapped w~ row (pk cols 152/153);
    # route them to slot 127 so no runtime patch of wrow is needed.
    replt2[24, 127] = 1.0
    replt2[25, 128 + 127] = 1.0
    replt2 = np.ascontiguousarray(replt2)
    m127_host = np.zeros((128, 1), np.uint8)
    m127_host[127, 0] = 1

    in_maps = []
    for c in range(NCORES):
        hA, hB = 2 * c, 2 * c + 1
        wallh = np.zeros((DM, WALLN), np.float32)
        wallh[:, 0:128] = qw[:, 128 * c:128 * (c + 1)]
        wallh[:, 128:192] = vw[:, 128 * c:128 * c + 64]
        wallh[:, 194:258] = vw[:, 128 * c + 64:128 * (c + 1)]
        wallh[:, 260:388] = kw[:, 128 * c:128 * (c + 1)]
        wallh[:, 388:400] = lw[:, 12 * hA:12 * hA + 12]
        wallh[:, 400:412] = lw[:, 12 * hB:12 * hB + 12]
        colrowh = np.zeros((128, 1152), np.float32)
        colrowh[:, 0:128] = colind
        colrowh[:, 128:1152] = np.tile(rowind, (1, NB))
        biasfh = np.zeros((128, 3), np.float32)
        biasfh[:, 0] = qb[128 * c:128 * (c + 1)]
        biasfh[:, 1] = qb[128 * c:128 * (c + 1)] + 1.0
        biasfh[127, 2] = 1.0
        bias1h = np.zeros((128, 284), np.float32)
        bias1h[:, 0:64] = vb[128 * c:128 * c + 64]
        bias1h[:, 66:130] = vb[128 * c + 64:128 * (c + 1)]
        bias1h[:, 132:260] = kb[128 * c:128 * (c + 1)]
        bias1h[:, 260:272] = lb[12 * hA:12 * hA + 12]
        bias1h[:, 272:284] = lb[12 * hB:12 * hB + 12]
        in_maps.append({
            "xT": xT,
            "wall": np.ascontiguousarray(wallh.astype(bf)),
            "owd": np.ascontiguousarray(
                ow[128 * c:128 * (c + 1), :].astype(bf)),
            "colrow": colrowh.astype(bf),
            "replt": replt2.astype(bf),
            "biasf": biasfh,
            "m127": m127_host,
            "bias1": bias1h,
        })
    with_bias = bool(np.any(vb) or np.any(kb) or np.any(lb))
    return in_maps, with_bias


def kernel(**inputs) -> np.ndarray:
    in_maps, with_bias = _host_prep(inputs)
    if with_bias not in _PROGRAM_CACHE:
        _PROGRAM_CACHE[with_bias] = _build_program(with_bias)
    nc = _PROGRAM_CACHE[with_bias]
    res = run_bass_kernel_spmd(nc, in_maps, list(range(NCORES)))
    ob = np.asarray(inputs["ob"], np.float32)
    out = np.zeros((B * T, DM), np.float32)
    for r in res.results:
        out += np.asarray(r["out"], np.float32)
    out += ob[None, :]
    return out.reshape(B, T, DM)
